# revision 1
# baseline (speedup 1.0000x reference)
"""Bilinear causal attention (nn_Attention_34772055228779) on 8 trn2 cores.

reference:
  scores[i,k] = x[i] @ W_bi[k] @ x[i]          [512, 512]
  attn = softmax(scores + causal_mask, axis=1)
  out  = (attn @ x) @ W_out.T                  [512, 512]

Device strategy (tensor-parallel over score columns, per sharding hint):
  core m holds W_bi[64m:64(m+1)]  (64 MiB fp32)
  stage A: for each local k: Y_k = X @ W_k  (fp32r matmuls, lhsT = X^T resident)
           scores[:, k] = rowsum(Y_k * X)   (fused DVE scalar_tensor_tensor)
  AllToAll over the [8 x 64-row, 64-col] score shard blocks: core m ends up
           with rows [64m, 64m+64) of the FULL score matrix.
  tail:    masked softmax rows (ACT exp with fused accum), A^T via PE
           transpose, O^T = X^T A^T, Y = O @ W_out^T, DMA 64 output rows.
  host:    concatenates the 8 row blocks.
"""
import numpy as np

N_CTX = 512
D = 512
NCORES = 8
KSH = N_CTX // NCORES      # 64 score columns per core
RSH = N_CTX // NCORES      # 64 output rows per core
NEG_INF = -1e30
STAGE_A = "causal"   # "causal" skips fully-masked row-tiles (k-interleaved)

_nc_cache = None


def _build(timing_loop=0, use_collective=True, num_devices=NCORES,
           stage_a="base", wbufs=4, wgroup=1):
    """Build the Bass module.

    timing_loop=R>0 wraps the whole per-core body in a hardware For_i loop
    (R iterations) for slope timing; collectives can't sit in control flow,
    so timing variants pass use_collective=False (the gather DMA then reads
    the pre-collective buffer -- wrong data, identical shapes/costs).
    """
    import concourse.mybir as mybir
    import concourse.tile as tile
    from concourse import bacc

    f32 = mybir.dt.float32
    f32r = mybir.dt.float32r
    Alu = mybir.AluOpType
    Act = mybir.ActivationFunctionType

    nc = bacc.Bacc(
        "TRN2", target_bir_lowering=False, debug=False,
        enable_asserts=False, num_devices=num_devices,
    )

    x_t = nc.dram_tensor("x", [N_CTX, D], f32, kind="ExternalInput").ap()
    # column-permuted X rows (k-interleaved layout) for the attn @ X matmul
    xp_t = nc.dram_tensor("xperm", [N_CTX, D], f32, kind="ExternalInput").ap()
    xt_t = nc.dram_tensor("xt", [D, N_CTX], f32, kind="ExternalInput").ap()
    wbi_t = nc.dram_tensor("wbi", [KSH, D, D], f32, kind="ExternalInput").ap()
    woutt_t = nc.dram_tensor("wout_t", [D, D], f32, kind="ExternalInput").ap()
    mask_t = nc.dram_tensor("mask", [RSH, N_CTX], f32, kind="ExternalInput").ap()
    ident_t = nc.dram_tensor("ident", [128, 128], f32, kind="ExternalInput").ap()
    out_t = nc.dram_tensor("out", [RSH, D], f32, kind="ExternalOutput").ap()

    with tile.TileContext(nc) as tc:
        with (
            tc.tile_pool(name="const", bufs=1) as cpool,
            tc.tile_pool(name="wstream", bufs=wbufs) as wpool,
            tc.tile_pool(name="scratch", bufs=3) as spool,
            tc.tile_pool(name="small", bufs=1) as mpool,
            tc.tile_pool(name="psA", bufs=6, space="PSUM") as ppA,
            tc.tile_pool(name="psB", bufs=2, space="PSUM") as ppB,
            tc.tile_pool(name="dram", bufs=1, space="DRAM") as dpool,
        ):
            # ---- resident loads (outside any timing loop) -----------------
            x_sb, xt_sb, woutt_sb = [], [], []
            for t in range(4):
                b = cpool.tile([128, N_CTX], f32r, tag=f"xt{t}", name=f"xt{t}")
                nc.sync.dma_start(b[:], xt_t[t * 128:(t + 1) * 128, :].bitcast(f32r))
                xt_sb.append(b)
            xp_sb = []
            for t in range(4):
                a = cpool.tile([128, N_CTX], f32, tag=f"x{t}", name=f"x{t}")
                nc.sync.dma_start(a[:], x_t[t * 128:(t + 1) * 128, :])
                x_sb.append(a)
                d = cpool.tile([128, N_CTX], f32, tag=f"xp{t}", name=f"xp{t}")
                xp_sb.append(d)
                c = cpool.tile([128, D], f32, tag=f"wo{t}", name=f"wo{t}")
                woutt_sb.append(c)
            mask_sb = cpool.tile([RSH, N_CTX], f32, tag="mask")
            ident_sb = cpool.tile([128, 128], f32, tag="ident")
            scores_sb = [
                cpool.tile([128, KSH], f32, tag=f"sc{t}", name=f"sc{t}")
                for t in range(4)
            ]
            if stage_a == "causal":
                # skipped (nt, kk) cells are never written; zero them so no
                # NaN bit-patterns survive into exp() past the additive mask
                for t in range(4):
                    nc.gpsimd.memset(scores_sb[t][:], 0.0)
            agin = dpool.tile([N_CTX, KSH], f32, tag="agin")
            agout = dpool.tile([N_CTX, KSH], f32, tag="agout")

            def load_wk(kk):
                wk = wpool.tile([128, 4, D], f32r, tag="wk", name="wk")
                nc.sync.dma_start(
                    wk[:],
                    wbi_t[kk].rearrange("(dt p) e -> p dt e", p=128).bitcast(f32r),
                )
                return wk

            def load_wk_group(kb):
                # one big DMA for `wgroup` consecutive local k's (better
                # HBM efficiency than 1 MiB transfers)
                wk = wpool.tile([128, wgroup * 4, D], f32r, tag="wk", name="wk")
                nc.sync.dma_start(
                    wk[:],
                    wbi_t[kb:kb + wgroup]
                    .rearrange("g (dt p) e -> p (g dt) e", p=128)
                    .bitcast(f32r),
                )
                return wk

            def emit_stt(yp, nt, kk):
                scr = spool.tile([128, D], f32, tag="stt_out", name="scr")
                nc.vector.scalar_tensor_tensor(
                    out=scr[:], in0=yp[:], scalar=1.0, in1=x_sb[nt][:],
                    op0=Alu.mult, op1=Alu.mult,
                    accum_out=scores_sb[nt][:, kk:kk + 1],
                )

            def stage_a_base():
                # causal: with k-interleaved sharding (global k = 8*kk + m),
                # row-tiles nt < kk//16 are fully masked for column kk on
                # EVERY core, so the skip bound is SPMD-uniform.
                for kb in range(0, KSH, wgroup):
                    wk = load_wk_group(kb) if wgroup > 1 else load_wk(kb)
                    for g in range(wgroup):
                        kk = kb + g
                        nt_lo = (kk // 16) if stage_a == "causal" else 0
                        for nt in range(nt_lo, 4):
                            yp = ppA.tile([128, D], f32, tag="yp", name="yp")
                            for dt in range(4):
                                nc.tensor.matmul(
                                    yp[:],
                                    lhsT=xt_sb[dt][:, nt * 128:(nt + 1) * 128],
                                    rhs=wk[:, g * 4 + dt, :],
                                    start=(dt == 0),
                                    stop=(dt == 3),
                                )
                            emit_stt(yp, nt, kk)

            def stage_a_kpair():
                # process k in pairs; consecutive matmuls share the same
                # stationary lhsT tile (halves PE weight reloads)
                for kk in range(0, KSH, 2):
                    wk0 = load_wk(kk)
                    wk1 = load_wk(kk + 1)
                    for nt in range(4):
                        yp0 = ppA.tile([128, D], f32, tag="yp", name="yp")
                        yp1 = ppA.tile([128, D], f32, tag="yp", name="yp")
                        for dt in range(4):
                            lhsT = xt_sb[dt][:, nt * 128:(nt + 1) * 128]
                            nc.tensor.matmul(
                                yp0[:], lhsT=lhsT, rhs=wk0[:, dt, :],
                                start=(dt == 0), stop=(dt == 3),
                                skip_group_check=True)
                            nc.tensor.matmul(
                                yp1[:], lhsT=lhsT, rhs=wk1[:, dt, :],
                                start=(dt == 0), stop=(dt == 3),
                                skip_group_check=True)
                        emit_stt(yp0, nt, kk)
                        emit_stt(yp1, nt, kk + 1)

            def body():
                # ---- stage A: local score columns -------------------------
                if stage_a == "kpair":
                    stage_a_kpair()
                else:
                    stage_a_base()

                # tail-only constants: emitted after stage A so their DMAs
                # don't delay the first W_k prefetches
                nc.sync.dma_start(mask_sb[:], mask_t[:])
                nc.sync.dma_start(ident_sb[:], ident_t[:])
                for t in range(4):
                    nc.sync.dma_start(
                        woutt_sb[t][:], woutt_t[t * 128:(t + 1) * 128, :])
                    nc.sync.dma_start(
                        xp_sb[t][:], xp_t[t * 128:(t + 1) * 128, :])

                # ---- AllToAll: shard columns -> shard rows ----------------
                for nt in range(4):
                    nc.sync.dma_start(
                        agin[nt * 128:(nt + 1) * 128, :], scores_sb[nt][:])
                if use_collective:
                    nc.gpsimd.collective_compute(
                        "AllToAll",
                        mybir.AluOpType.bypass,
                        replica_groups=[list(range(NCORES))],
                        ins=[agin[:].opt()],
                        outs=[agout[:].opt()],
                    )
                    coll_out = agout
                else:
                    coll_out = agin
                # rows of the full score matrix for this core: [64, 512]
                sfull = mpool.tile([RSH, N_CTX], f32, tag="sfull", name="sfull")
                nc.sync.dma_start(
                    sfull[:].rearrange("i (r k) -> i r k", r=NCORES),
                    coll_out[:].rearrange("(r i) k -> i r k", r=NCORES),
                )

                # ---- masked softmax over the 64 rows ----------------------
                sm = mpool.tile([RSH, N_CTX], f32, tag="sm", name="sm")
                nc.vector.tensor_tensor(
                    out=sm[:], in0=sfull[:], in1=mask_sb[:], op=Alu.add)
                negm = mpool.tile([RSH, 1], f32, tag="negm", name="negm")
                nc.vector.reduce_max(negm[:], sm[:], axis=mybir.AxisListType.X,
                                     negate=True)
                esb = mpool.tile([RSH, N_CTX], f32, tag="esb", name="esb")
                den = mpool.tile([RSH, 1], f32, tag="den", name="den")
                nc.scalar.activation(
                    esb[:], sm[:], Act.Exp, bias=negm[:], scale=1.0,
                    accum_out=den[:])
                rden = mpool.tile([RSH, 1], f32, tag="rden", name="rden")
                nc.vector.reciprocal(rden[:], den[:])
                a_sb = mpool.tile([RSH, N_CTX], f32, tag="a_sb", name="a_sb")
                nc.vector.tensor_scalar_mul(a_sb[:], esb[:], rden[:])

                # ---- A^T via PE transpose: [64, 512] -> 4x [128, 64] ------
                at_sb = []
                for kt in range(4):
                    tp = ppB.tile([128, 512], f32, tag="tail", name="tp")
                    nc.tensor.transpose(
                        tp[:, 0:RSH],
                        a_sb[:, kt * 128:(kt + 1) * 128],
                        ident_sb[0:RSH, 0:RSH],
                    )
                    at = mpool.tile([128, RSH], f32, tag=f"at{kt}", name=f"at{kt}")
                    nc.scalar.copy(at[:], tp[:, 0:RSH])
                    at_sb.append(at)

                # ---- O^T = X^T @ A^T : [512(e), 64(i)] --------------------
                ot_sb = []
                for et in range(4):
                    op = ppB.tile([128, 512], f32, tag="tail", name="op")
                    for kt in range(4):
                        nc.tensor.matmul(
                            op[:, 0:RSH],
                            lhsT=xp_sb[kt][:, et * 128:(et + 1) * 128],
                            rhs=at_sb[kt][:],
                            start=(kt == 0),
                            stop=(kt == 3),
                        )
                    ot = mpool.tile([128, RSH], f32, tag=f"ot{et}", name=f"ot{et}")
                    nc.scalar.copy(ot[:], op[:, 0:RSH])
                    ot_sb.append(ot)

                # ---- Y = O @ W_out^T : [64(i), 512(f)] --------------------
                ypz = ppB.tile([128, 512], f32, tag="tail", name="ypz")
                for et in range(4):
                    nc.tensor.matmul(
                        ypz[0:RSH, :],
                        lhsT=ot_sb[et][:],
                        rhs=woutt_sb[et][:],
                        start=(et == 0),
                        stop=(et == 3),
                    )
                y_sb = mpool.tile([RSH, D], f32, tag="y_sb", name="y_sb")
                nc.scalar.copy(y_sb[:], ypz[0:RSH, :])
                nc.sync.dma_start(out_t[:], y_sb[:])

            if timing_loop:
                with tc.For_i(0, timing_loop, 1):
                    body()
            else:
                body()

    nc.compile()
    return nc


def _make_in_maps(x, W_bi, W_out, stage_a="causal"):
    x = np.ascontiguousarray(np.asarray(x, dtype=np.float32))
    W_bi = np.asarray(W_bi, dtype=np.float32)
    W_out = np.asarray(W_out, dtype=np.float32)
    xt = np.ascontiguousarray(x.T)
    woutt = np.ascontiguousarray(W_out.T)
    ident = np.eye(128, dtype=np.float32)
    if stage_a == "causal":
        # interleaved k-sharding: core m owns global columns k = 8*kk + m.
        # After the AllToAll gather, score column position p = r*64 + kk
        # holds global k = 8*kk + r, so X rows and the causal mask are
        # permuted to match.
        perm = np.array([8 * (p % KSH) + p // KSH for p in range(N_CTX)])
        xperm = np.ascontiguousarray(x[perm])
        kcol = perm[None, :]                       # global k at position p
        shards = [np.ascontiguousarray(W_bi[m::NCORES]) for m in range(NCORES)]
    else:
        perm = np.arange(N_CTX)
        xperm = x
        kcol = perm[None, :]
        shards = [np.ascontiguousarray(W_bi[m * KSH:(m + 1) * KSH])
                  for m in range(NCORES)]
    in_maps = []
    for m in range(NCORES):
        rows = np.arange(m * RSH, (m + 1) * RSH)[:, None]
        mask = np.where(kcol <= rows, 0.0, NEG_INF).astype(np.float32)
        in_maps.append({
            "x": x,
            "xperm": xperm,
            "xt": xt,
            "wbi": shards[m],
            "wout_t": woutt,
            "mask": np.ascontiguousarray(mask),
            "ident": ident,
        })
    return in_maps


def kernel(x, W_bi, W_out):
    global _nc_cache
    import time as _time
    from concourse.bass_utils import run_bass_kernel_spmd

    if _nc_cache is None:
        _nc_cache = _build(stage_a=STAGE_A)
    nc = _nc_cache
    in_maps = _make_in_maps(x, W_bi, W_out, stage_a=STAGE_A)
    last_exc = None
    for attempt in range(3):
        try:
            res = run_bass_kernel_spmd(nc, in_maps, core_ids=list(range(NCORES)),
                                       trace=False)
            break
        except Exception as e:  # transient NRT/axon wedges recover on retry
            last_exc = e
            _time.sleep(5.0 * (attempt + 1))
    else:
        raise last_exc
    out = np.concatenate([res.results[m]["out"] for m in range(NCORES)], axis=0)
    return np.ascontiguousarray(out, dtype=np.float32)



# revision 2
# speedup vs baseline: 1.8701x; 1.8701x over previous
"""Bilinear causal attention (nn_Attention_34772055228779) on 8 trn2 cores.

reference:
  scores[i,k] = x[i] @ W_bi[k] @ x[i]          [512, 512]
  attn = softmax(scores + causal_mask, axis=1)
  out  = (attn @ x) @ W_out.T                  [512, 512]

Key algebra: scores depend only on the symmetric part of W_bi[k], so the
host folds W into an upper-triangular U_k = triu(W_k + W_k^T, 1) + diag(W_k)
with x^T U_k x == x^T W_k x.  Only the 10 (of 16) upper [128,128] tiles of
U_k are nonzero -> 0.625x PE work, and fp16 weights halve the bytes again:
21 MB of HBM traffic per core instead of 67 MB.

Device strategy (tensor-parallel over score columns, per sharding hint):
  core m holds U_k for global k = 8*kk + m (k-interleaved so the causal
  row-tile skip bound is SPMD-uniform).
  stage A per kk: Y = X @ U_k via 4 matmuls per surviving row-tile
           (N = 512/384/256/128 columns, triangular), PSUM tile [128,4,512].
           ACT casts Y -> fp16 SBUF; DVE scalar_tensor_tensor (fp16 2x mode)
           computes scores[:, kk] = rowsum(Y * X).
  AllToAll over the [8 x 64-row, 64-col] score shard blocks: core m ends up
           with rows [64m, 64m+64) of the FULL score matrix.
  tail:    masked softmax rows (ACT exp with fused accum), A^T via PE
           transpose, O^T = X^T A^T, Y = O @ W_out^T, DMA 64 output rows.
  host:    concatenates the 8 row blocks.
"""
import numpy as np

N_CTX = 512
D = 512
NCORES = 8
KSH = N_CTX // NCORES      # 64 score columns per core
RSH = N_CTX // NCORES      # 64 output rows per core
NEG_INF = -1e30
STAGE_A = "causal"

W_DT = "f16"               # "f16" | "bf16": matmul/weight dtype
MM_MODE = "mm4"            # "mm4": 4 wide MMs/pair | "mm10": 10x 128-col MMs
WBUFS = 8                  # W-stream prefetch depth

# tile (dt, et) pairs of the upper-triangular U, dt-major; woff[dt] = index
# of the first tile for row-block dt in the packed [128, 10*128] layout
SEL = [(0, 0), (0, 1), (0, 2), (0, 3), (1, 1), (1, 2), (1, 3),
       (2, 2), (2, 3), (3, 3)]
WOFF = [0, 4, 7, 9]

_nc_cache = None


def _build(timing_loop=0, use_collective=True, num_devices=NCORES,
           stage_a="causal", wbufs=WBUFS, wgroup=1):
    """Build the Bass module.

    timing_loop=R>0 wraps the whole per-core body in a hardware For_i loop
    (R iterations) for slope timing; collectives can't sit in control flow,
    so timing variants pass use_collective=False (the gather DMA then reads
    the pre-collective buffer -- wrong data, identical shapes/costs).
    """
    import concourse.mybir as mybir
    import concourse.tile as tile
    from concourse import bacc

    f32 = mybir.dt.float32
    f16 = mybir.dt.float16 if W_DT == "f16" else mybir.dt.bfloat16
    Alu = mybir.AluOpType
    Act = mybir.ActivationFunctionType

    nc = bacc.Bacc(
        "TRN2", target_bir_lowering=False, debug=False,
        enable_asserts=False, num_devices=num_devices,
    )

    # column-permuted X rows (k-interleaved layout) for the attn @ X matmul
    xp_t = nc.dram_tensor("xperm", [N_CTX, D], f32, kind="ExternalInput").ap()
    xt16_t = nc.dram_tensor("xt16", [D, N_CTX], f16, kind="ExternalInput").ap()
    xf16_t = nc.dram_tensor("xf16", [128, 4, D], f16, kind="ExternalInput").ap()
    wu_t = nc.dram_tensor("wu", [KSH, 128, 1280], f16, kind="ExternalInput").ap()
    woutt_t = nc.dram_tensor("wout_t", [D, D], f32, kind="ExternalInput").ap()
    mask_t = nc.dram_tensor("mask", [RSH, N_CTX], f32, kind="ExternalInput").ap()
    ident_t = nc.dram_tensor("ident", [128, 128], f32, kind="ExternalInput").ap()
    out_t = nc.dram_tensor("out", [RSH, D], f32, kind="ExternalOutput").ap()

    with tile.TileContext(nc) as tc:
        with (
            tc.tile_pool(name="const", bufs=1) as cpool,
            tc.tile_pool(name="wstream", bufs=wbufs) as wpool,
            tc.tile_pool(name="ycast", bufs=2) as ypool,
            tc.tile_pool(name="scratch", bufs=2) as spool,
            tc.tile_pool(name="small", bufs=1) as mpool,
            tc.tile_pool(name="psA", bufs=2, space="PSUM") as ppA,
            tc.tile_pool(name="dram", bufs=1, space="DRAM") as dpool,
        ):
            # ---- resident loads (outside any timing loop) -----------------
            xt_sb = []
            for t in range(4):
                b = cpool.tile([128, N_CTX], f16, tag=f"xt{t}", name=f"xt{t}")
                nc.sync.dma_start(b[:], xt16_t[t * 128:(t + 1) * 128, :])
                xt_sb.append(b)
            xf_sb = cpool.tile([128, 4, D], f16, tag="xf16")
            nc.sync.dma_start(xf_sb[:], xf16_t[:])
            xp_sb, woutt_sb = [], []
            for t in range(4):
                d = cpool.tile([128, N_CTX], f32, tag=f"xp{t}", name=f"xp{t}")
                xp_sb.append(d)
                c = cpool.tile([128, D], f32, tag=f"wo{t}", name=f"wo{t}")
                woutt_sb.append(c)
            mask_sb = cpool.tile([RSH, N_CTX], f32, tag="mask")
            ident_sb = cpool.tile([128, 128], f32, tag="ident")
            scores_sb = [
                cpool.tile([128, KSH], f32, tag=f"sc{t}", name=f"sc{t}")
                for t in range(4)
            ]
            # skipped (nt, kk) cells are never written; zero them so no
            # NaN bit-patterns survive into exp() past the additive mask
            for t in range(4):
                nc.gpsimd.memset(scores_sb[t][:], 0.0)
            agin = dpool.tile([N_CTX, KSH], f32, tag="agin")
            agout = dpool.tile([N_CTX, KSH], f32, tag="agout")

            def stage_a_col(kk):
                # nt tiles with all 128 rows < global k (=8*kk+m) are fully
                # masked on every core -> SPMD-uniform skip
                nt_lo = (kk // 16) if stage_a == "causal" else 0
                wk = wpool.tile([128, 10 * 128], f16, tag="wk", name="wk")
                nc.sync.dma_start(wk[:], wu_t[kk])
                ypk = ppA.tile([128, 4, D], f32, tag="ypk", name="ypk")
                for nt in range(nt_lo, 4):
                    for dt in range(4):
                        lhsT = xt_sb[dt][:, nt * 128:(nt + 1) * 128]
                        if MM_MODE == "mm4":
                            nc.tensor.matmul(
                                ypk[:, nt, dt * 128:512],
                                lhsT=lhsT,
                                rhs=wk[:, WOFF[dt] * 128:
                                       (WOFF[dt] + 4 - dt) * 128],
                                start=(dt == 0),
                                stop=(dt == 3),
                                skip_group_check=True,
                            )
                        else:
                            for et in range(dt, 4):
                                nc.tensor.matmul(
                                    ypk[:, nt, et * 128:(et + 1) * 128],
                                    lhsT=lhsT,
                                    rhs=wk[:, (WOFF[dt] + et - dt) * 128:
                                           (WOFF[dt] + et - dt + 1) * 128],
                                    start=(dt == 0),
                                    stop=(dt == et),
                                    skip_group_check=True,
                                )
                # ACT: cast the surviving row-tiles PSUM fp32 -> SBUF fp16
                ybf = ypool.tile([128, 4, D], f16, tag="ybf", name="ybf")
                nc.scalar.copy(ybf[:, nt_lo:4, :], ypk[:, nt_lo:4, :])
                # DVE: scores[:, kk] = rowsum(Y * X) in fp16 2x mode
                for nt in range(nt_lo, 4):
                    scr = spool.tile([128, D], f16, tag="stt_out", name="scr")
                    nc.vector.scalar_tensor_tensor(
                        out=scr[:], in0=ybf[:, nt, :], scalar=1.0,
                        in1=xf_sb[:, nt, :],
                        op0=Alu.mult, op1=Alu.mult,
                        accum_out=scores_sb[nt][:, kk:kk + 1],
                    )

            def body():
                # ---- stage A: local score columns -------------------------
                for kk in range(KSH):
                    stage_a_col(kk)

                # tail-only constants: emitted after stage A so their DMAs
                # don't delay the first W_k prefetches
                nc.sync.dma_start(mask_sb[:], mask_t[:])
                nc.sync.dma_start(ident_sb[:], ident_t[:])
                for t in range(4):
                    nc.sync.dma_start(
                        woutt_sb[t][:], woutt_t[t * 128:(t + 1) * 128, :])
                    nc.sync.dma_start(
                        xp_sb[t][:], xp_t[t * 128:(t + 1) * 128, :])

                # ---- AllToAll: shard columns -> shard rows ----------------
                for nt in range(4):
                    nc.sync.dma_start(
                        agin[nt * 128:(nt + 1) * 128, :], scores_sb[nt][:])
                if use_collective:
                    nc.gpsimd.collective_compute(
                        "AllToAll",
                        mybir.AluOpType.bypass,
                        replica_groups=[list(range(NCORES))],
                        ins=[agin[:].opt()],
                        outs=[agout[:].opt()],
                    )
                    coll_out = agout
                else:
                    coll_out = agin
                # rows of the full score matrix for this core: [64, 512]
                sfull = mpool.tile([RSH, N_CTX], f32, tag="sfull", name="sfull")
                nc.sync.dma_start(
                    sfull[:].rearrange("i (r k) -> i r k", r=NCORES),
                    coll_out[:].rearrange("(r i) k -> i r k", r=NCORES),
                )

                # ---- masked softmax over the 64 rows ----------------------
                sm = mpool.tile([RSH, N_CTX], f32, tag="sm", name="sm")
                nc.vector.tensor_tensor(
                    out=sm[:], in0=sfull[:], in1=mask_sb[:], op=Alu.add)
                negm = mpool.tile([RSH, 1], f32, tag="negm", name="negm")
                nc.vector.reduce_max(negm[:], sm[:], axis=mybir.AxisListType.X,
                                     negate=True)
                esb = mpool.tile([RSH, N_CTX], f32, tag="esb", name="esb")
                den = mpool.tile([RSH, 1], f32, tag="den", name="den")
                nc.scalar.activation(
                    esb[:], sm[:], Act.Exp, bias=negm[:], scale=1.0,
                    accum_out=den[:])
                rden = mpool.tile([RSH, 1], f32, tag="rden", name="rden")
                nc.vector.reciprocal(rden[:], den[:])
                a_sb = mpool.tile([RSH, N_CTX], f32, tag="a_sb", name="a_sb")
                nc.vector.tensor_scalar_mul(a_sb[:], esb[:], rden[:])

                # ---- A^T via PE transpose: [64, 512] -> 4x [128, 64] ------
                # tail PSUM reuses the stage-A pool's [128,4,512] tiles, one
                # bank per transpose / per O^T accumulation group
                tpb = ppA.tile([128, 4, D], f32, tag="ypk", name="tpb")
                at_sb = []
                for kt in range(4):
                    nc.tensor.transpose(
                        tpb[:, kt, 0:RSH],
                        a_sb[:, kt * 128:(kt + 1) * 128],
                        ident_sb[0:RSH, 0:RSH],
                    )
                    at = mpool.tile([128, RSH], f32, tag=f"at{kt}", name=f"at{kt}")
                    nc.scalar.copy(at[:], tpb[:, kt, 0:RSH])
                    at_sb.append(at)

                # ---- O^T = X^T @ A^T : [512(e), 64(i)] --------------------
                opb = ppA.tile([128, 4, D], f32, tag="ypk", name="opb")
                ot_sb = []
                for et in range(4):
                    for kt in range(4):
                        nc.tensor.matmul(
                            opb[:, et, 0:RSH],
                            lhsT=xp_sb[kt][:, et * 128:(et + 1) * 128],
                            rhs=at_sb[kt][:],
                            start=(kt == 0),
                            stop=(kt == 3),
                        )
                    ot = mpool.tile([128, RSH], f32, tag=f"ot{et}", name=f"ot{et}")
                    nc.scalar.copy(ot[:], opb[:, et, 0:RSH])
                    ot_sb.append(ot)

                # ---- Y = O @ W_out^T : [64(i), 512(f)] --------------------
                zpb = ppA.tile([128, 4, D], f32, tag="ypk", name="zpb")
                for et in range(4):
                    nc.tensor.matmul(
                        zpb[0:RSH, 0, :],
                        lhsT=ot_sb[et][:],
                        rhs=woutt_sb[et][:],
                        start=(et == 0),
                        stop=(et == 3),
                    )
                y_sb = mpool.tile([RSH, D], f32, tag="y_sb", name="y_sb")
                nc.scalar.copy(y_sb[:], zpb[0:RSH, 0, :])
                nc.sync.dma_start(out_t[:], y_sb[:])

            if timing_loop:
                with tc.For_i(0, timing_loop, 1):
                    body()
            else:
                body()

    nc.compile()
    return nc


def _np_w_dtype():
    if W_DT == "f16":
        return np.float16
    import ml_dtypes
    return ml_dtypes.bfloat16


def _make_in_maps(x, W_bi, W_out, stage_a="causal"):
    x = np.ascontiguousarray(np.asarray(x, dtype=np.float32))
    W_bi = np.asarray(W_bi, dtype=np.float32)
    W_out = np.asarray(W_out, dtype=np.float32)
    wdt = _np_w_dtype()
    xt16 = np.ascontiguousarray(x.T).astype(wdt)
    xf16 = np.ascontiguousarray(
        x.reshape(4, 128, D).transpose(1, 0, 2)).astype(wdt)
    woutt = np.ascontiguousarray(W_out.T)
    ident = np.eye(128, dtype=np.float32)

    # fold W -> upper-triangular U (x^T U x == x^T W x)
    Wsym = W_bi + W_bi.transpose(0, 2, 1)
    U = np.triu(Wsym, 1)
    idx = np.arange(N_CTX)
    U[:, idx, idx] = W_bi[:, idx, idx]

    # interleaved k-sharding: core m owns global columns k = 8*kk + m.
    # After the AllToAll gather, score column position p = r*64 + kk
    # holds global k = 8*kk + r, so X rows and the causal mask are
    # permuted to match.
    perm = np.array([8 * (p % KSH) + p // KSH for p in range(N_CTX)])
    xperm = np.ascontiguousarray(x[perm])
    kcol = perm[None, :]                       # global k at position p

    in_maps = []
    for m in range(NCORES):
        shard = U[m::NCORES]                   # [64, 512, 512]
        V = shard.reshape(KSH, 4, 128, 4, 128)  # [k, dt, p, et, c]
        wu = np.stack([V[:, dt, :, et, :] for (dt, et) in SEL], axis=2)
        wu = np.ascontiguousarray(wu.transpose(0, 1, 2, 3).reshape(
            KSH, 128, 10 * 128)).astype(wdt)
        rows = np.arange(m * RSH, (m + 1) * RSH)[:, None]
        mask = np.where(kcol <= rows, 0.0, NEG_INF).astype(np.float32)
        in_maps.append({
            "xperm": xperm,
            "xt16": xt16,
            "xf16": xf16,
            "wu": wu,
            "wout_t": woutt,
            "mask": np.ascontiguousarray(mask),
            "ident": ident,
        })
    return in_maps


def kernel(x, W_bi, W_out):
    global _nc_cache
    import time as _time
    from concourse.bass_utils import run_bass_kernel_spmd

    if _nc_cache is None:
        _nc_cache = _build(stage_a=STAGE_A)
    nc = _nc_cache
    in_maps = _make_in_maps(x, W_bi, W_out, stage_a=STAGE_A)
    last_exc = None
    for attempt in range(3):
        try:
            res = run_bass_kernel_spmd(nc, in_maps, core_ids=list(range(NCORES)),
                                       trace=False)
            break
        except Exception as e:  # transient NRT/axon wedges recover on retry
            last_exc = e
            _time.sleep(5.0 * (attempt + 1))
    else:
        raise last_exc
    out = np.concatenate([res.results[m]["out"] for m in range(NCORES)], axis=0)
    return np.ascontiguousarray(out, dtype=np.float32)
